# revision 17
# baseline (speedup 1.0000x reference)
"""Trainium2 Bass kernel for nn_Loss_4861902979528.

Computes, for embeddings [N,D] and adj [N,N]:
    e      = embeddings / max(||row||_4, 1e-12)
    log_p  = log(e + 1e-10)
    kl     = p_log_p[:,None] - e @ log_p.T
    adj_n  = adj / max(row_l1(adj), 1e-12)
    out    = lambda * sum(kl * adj_n)

Restructured to avoid materializing any [N,N] intermediate:
    out = lam * ( sum_i plp[i]*rsn[i]  -  sum_{j,d} log_p[j,d] * V[j,d] )
    V[j,d] = sum_i adj[i,j] * recip[i] * e[i,d]     (PE matmul, adj streamed once)
    rsn[i] = rowsum(adj)[i] * recip[i]
    recip[i] = 1/max(rowsum(adj)[i], 1e-12)

Sharding: row-blocks of adj across 8 cores (each core owns N/8 = 1536 rows).
Each core computes partial scalars; host sums the partials (scalar all-reduce
done on host) and applies lambda.
"""

import numpy as np

import concourse.bass as bass
import concourse.tile as tile
import concourse.mybir as mybir
from concourse.bass_utils import run_bass_kernel_spmd
from concourse.masks import make_identity

N = 12288          # rows/cols of adj; rows of embeddings
D = 128            # embedding dim
NCORES = 8
R = N // NCORES    # 1536 rows of adj per core
NCH = R // 128     # 12 own 128-row chunks per core
NFULL = N // 128   # 96 chunks in full embeddings
JW = 512           # j-window (one PSUM bank of fp32)
NJW = N // JW      # 24 windows
SB = 2             # adj row-blocks per super-block (PSUM accumulation depth)
NSB = NCH // SB    # 6 super-blocks
EPS_NORM = 1e-12
EPS_LOG = 1e-10

F32 = mybir.dt.float32
F32R = mybir.dt.float32r
BF16 = mybir.dt.bfloat16
AF = mybir.ActivationFunctionType
ALU = mybir.AluOpType

# The V-matmul runs in bf16: the ACT pass that computes row sums doubles as
# a cast, writing the bf16 copy of each adj block into the first half of the
# same SBUF buffer (bf16 writes at byte 2i trail f32 reads at byte 4i, so
# the in-place overlap is safe). bf16 streams through the PE at 1 cyc/row
# (fp32 would be 4) and the contraction over 1536 rows averages the rounding
# noise to ~1e-5 relative on the final scalar.

# Fused multiply+reduce on DVE. "stt" uses scalar_tensor_tensor with
# accum_out (1 DVE op); "2op" falls back to tensor_mul + reduce_sum.
MUL_REDUCE_MODE = "stt"


def _mul_reduce(nc, out_scratch, in0, in1, accum_col):
    if MUL_REDUCE_MODE == "stt":
        nc.vector.scalar_tensor_tensor(
            out=out_scratch, in0=in0, scalar=1.0, in1=in1,
            op0=ALU.mult, op1=ALU.mult, accum_out=accum_col,
        )
    else:
        nc.vector.tensor_mul(out_scratch, in0, in1)
        nc.vector.reduce_sum(accum_col, out_scratch, axis=mybir.AxisListType.X)


def _split_excess_waits(nc: bass.Bass, max_waits: int = 1) -> None:
    """This walrus build rejects instructions carrying more than a couple of
    semaphore waits ("Too many sync wait commands"). Hoist excess waits onto
    same-engine NOPs inserted just before the offending instruction."""
    n_split = 0
    for fn in nc.m.functions:
        for bb in fn.blocks:
            insts = bb.instructions
            out = []
            changed = False
            for inst in insts:
                si = inst.sync_info
                waits = list(si.on_wait) if si is not None and si.on_wait else []
                if len(waits) > max_waits:
                    extra, keep = waits[:-max_waits], waits[-max_waits:]
                    for i in range(0, len(extra), max_waits):
                        n_split += 1
                        out.append(
                            mybir.InstNoOp(
                                name=f"{inst.name}-ws{i}",
                                engine=inst.engine,
                                sync_info=mybir.SyncInfo(
                                    on_wait=extra[i : i + max_waits], on_update=[]
                                ),
                                bass_nofuse=True,
                            )
                        )
                    inst.sync_info = mybir.SyncInfo(
                        on_wait=keep,
                        on_update=list(si.on_update) if si.on_update else [],
                    )
                    changed = True
                out.append(inst)
            if changed:
                bb.instructions = out


def build_program() -> bass.Bass:
    nc = bass.Bass()

    adj = nc.declare_dram_parameter("adj_block", [R, N], F32, isOutput=False)
    emb = nc.declare_dram_parameter("emb", [N, D], F32, isOutput=False)
    emb_own = nc.declare_dram_parameter("emb_own", [R, D], F32, isOutput=False)
    acc1_d = nc.declare_dram_parameter("acc1", [128, NCH], F32, isOutput=True)
    acc2_d = nc.declare_dram_parameter("acc2", [128, NSB * NJW], F32, isOutput=True)

    with tile.TileContext(nc) as tc:
        with (
            tc.tile_pool(name="blk", bufs=3) as blk_pool,
            tc.tile_pool(name="persist", bufs=1) as persist,
            tc.tile_pool(name="chunk", bufs=3) as chunk_pool,
            tc.tile_pool(name="sq", bufs=2) as sq_pool,
            tc.tile_pool(name="trash", bufs=2) as trash_pool,
            tc.tile_pool(name="e2", bufs=3) as e2_pool,
            tc.tile_pool(name="psum_v", bufs=6, space="PSUM") as psum_v,
            tc.tile_pool(name="psum_t", bufs=2, space="PSUM") as psum_t,
        ):
            # ---- persistent small tiles ----
            logpT = persist.tile([128, N], BF16, tag="logpT")   # log_p transposed [d, j]
            e_own = persist.tile([128, R], F32, tag="e_own")    # normalized e, own rows
            embo = persist.tile([128, NCH, D], F32, tag="embo")
            ident = persist.tile([128, 128], F32, tag="ident")
            s4f = persist.tile([128, NFULL], F32, tag="s4f")
            rnf = persist.tile([128, NFULL], F32, tag="rnf")
            s4o = persist.tile([128, NCH], F32, tag="s4o")
            rno = persist.tile([128, NCH], F32, tag="rno")
            plp = persist.tile([128, NCH], F32, tag="plp")      # sum_d e*log_p per own row
            rs = persist.tile([128, NCH], F32, tag="rs")        # adj row sums
            rsc = persist.tile([128, NCH], F32, tag="rsc")      # max(rs, eps)
            rc = persist.tile([128, NCH], F32, tag="rc")        # 1/max(rs, eps)
            acc1_t = persist.tile([128, NCH], F32, tag="acc1")
            acc2_t = persist.tile([128, NSB * NJW], F32, tag="acc2")
            epsl = persist.tile([128, 1], F32, tag="epsl")

            make_identity(nc, ident)
            nc.vector.memset(epsl, EPS_LOG)

            # ================= prep: own rows (plp, e_own) =================
            nc.sync.dma_start(
                out=embo, in_=emb_own.rearrange("(n p) d -> p n d", p=128)
            )
            for oc in range(NCH):
                src = embo[:, oc, :]
                sq = sq_pool.tile([128, D], F32)
                nc.vector.tensor_mul(sq, src, src)
                tr = trash_pool.tile([128, D], F32)
                _mul_reduce(nc, tr, sq, sq, s4o[:, oc : oc + 1])
            # rnorm = (max(s4, eps^4))^(-1/4) via exp(-ln(s4)/4)
            nc.vector.tensor_scalar_max(s4o, s4o, EPS_NORM**4)
            nc.scalar.activation(rno, s4o, AF.Ln)
            nc.scalar.activation(rno, rno, AF.Exp, scale=-0.25)
            for oc in range(NCH):
                esl = e_own[:, oc * 128 : (oc + 1) * 128]
                nc.vector.tensor_scalar_mul(esl, embo[:, oc, :], rno[:, oc : oc + 1])
                lg = chunk_pool.tile([128, D], F32)
                nc.scalar.activation(lg, esl, AF.Ln, bias=epsl)
                tr = trash_pool.tile([128, D], F32)
                _mul_reduce(nc, tr, esl, lg, plp[:, oc : oc + 1])

            # ================= prep: full embeddings -> logpT =================
            embf = blk_pool.tile([128, N], F32, tag="blk")
            nc.sync.dma_start(
                out=embf.rearrange("p (n d) -> p n d", d=D),
                in_=emb.rearrange("(n p) d -> p n d", p=128),
            )
            for n in range(NFULL):
                src = embf[:, n * 128 : (n + 1) * 128]
                sq = sq_pool.tile([128, D], F32)
                nc.vector.tensor_mul(sq, src, src)
                tr = trash_pool.tile([128, D], F32)
                _mul_reduce(nc, tr, sq, sq, s4f[:, n : n + 1])
            nc.vector.tensor_scalar_max(s4f, s4f, EPS_NORM**4)
            nc.scalar.activation(rnf, s4f, AF.Ln)
            nc.scalar.activation(rnf, rnf, AF.Exp, scale=-0.25)
            for n in range(NFULL):
                ec = chunk_pool.tile([128, 128], F32)
                nc.vector.tensor_scalar_mul(
                    ec, embf[:, n * 128 : (n + 1) * 128], rnf[:, n : n + 1]
                )
                pt = psum_t.tile([128, 128], F32)
                nc.tensor.transpose(pt, ec, ident)
                nc.scalar.activation(
                    logpT[:, n * 128 : (n + 1) * 128], pt, AF.Ln, bias=epsl
                )

            # ================= hot loop: stream adj row blocks =================
            for sb in range(NSB):
                blks = []
                e2s = []
                for k in range(SB):
                    ic = sb * SB + k
                    blk = blk_pool.tile([128, N], F32, tag="blk")
                    nc.sync.dma_start(
                        out=blk, in_=adj[ic * 128 : (ic + 1) * 128, :]
                    )
                    # row sums via ACT copy with accumulate; the copy also
                    # casts the block to bf16, packed into the first half of
                    # the same buffer.
                    nc.scalar.activation(
                        blk.bitcast(BF16)[:, 0:N],
                        blk,
                        AF.Copy,
                        accum_out=rs[:, ic : ic + 1],
                    )
                    nc.vector.tensor_scalar_max(
                        rsc[:, ic : ic + 1], rs[:, ic : ic + 1], EPS_NORM
                    )
                    nc.vector.reciprocal(rc[:, ic : ic + 1], rsc[:, ic : ic + 1])
                    e2 = e2_pool.tile([128, 128], BF16)
                    nc.vector.tensor_scalar_mul(
                        e2, e_own[:, ic * 128 : (ic + 1) * 128], rc[:, ic : ic + 1]
                    )
                    blks.append(blk)
                    e2s.append(e2)
                for jw in range(NJW):
                    pv = psum_v.tile([128, JW], F32)
                    for k in range(SB):
                        nc.tensor.matmul(
                            pv,
                            e2s[k],
                            blks[k].bitcast(BF16)[:, jw * JW : (jw + 1) * JW],
                            start=(k == 0),
                            stop=(k == SB - 1),
                        )
                    _mul_reduce(
                        nc, pv, pv, logpT[:, jw * JW : (jw + 1) * JW],
                        acc2_t[:, sb * NJW + jw : sb * NJW + jw + 1],
                    )

            # term1 = plp * rs * rc  (rs*rc = rowsum of normalized adj, ~= 1)
            nc.vector.tensor_mul(acc1_t, rs, rc)
            nc.vector.tensor_mul(acc1_t, acc1_t, plp)

            nc.sync.dma_start(out=acc1_d[:, :], in_=acc1_t)
            nc.sync.dma_start(out=acc2_d[:, :], in_=acc2_t)

    _split_excess_waits(nc)
    return nc


_PROGRAM = None


def _get_program():
    global _PROGRAM
    if _PROGRAM is None:
        _PROGRAM = build_program()
    return _PROGRAM


def kernel(embeddings: np.ndarray, adj: np.ndarray, lambda_reg: np.ndarray) -> np.ndarray:
    embeddings = np.ascontiguousarray(np.asarray(embeddings, dtype=np.float32))
    adj = np.asarray(adj, dtype=np.float32)

    nc = _get_program()
    in_maps = []
    for c in range(NCORES):
        in_maps.append(
            {
                "adj_block": np.ascontiguousarray(adj[c * R : (c + 1) * R, :]),
                "emb": embeddings,
                "emb_own": np.ascontiguousarray(embeddings[c * R : (c + 1) * R, :]),
            }
        )
    out = run_bass_kernel_spmd(nc, in_maps, core_ids=list(range(NCORES)))
    total = np.float64(0.0)
    for r in out.results:
        total += r["acc1"].sum(dtype=np.float64) - r["acc2"].sum(dtype=np.float64)
    lam = np.float64(np.asarray(lambda_reg, dtype=np.float32))
    return np.asarray(np.float32(lam * total))
